# revision 2
# baseline (speedup 1.0000x reference)
"""Sliding-window local attention (KeOps ranges) on 8 Trainium2 cores.

Problem: B=4 H=16 T=4096 D=64, query block w=128 attends keys
[128(i-1), 128(i+1)) clamped to [0, T).  Softmax over the 256-key window,
out = attn @ V.  Only block 0 has out-of-range keys (its lower half), so
masking reduces to skipping that half-block.

Sharding: batch*head (64 pairs) split across 8 cores, 8 heads per core.

v2 design (per-core, all matmuls bf16, fp32 PSUM):
  - Scores S^T[k, q] = K_blk @ Q_blk^T, key-major, d=64 contraction on
    partitions [0:64] (head A) / [64:128] (head B).  The two heads' matmuls
    are emitted INTERLEAVED so the PE's 64x128 row-tiles (T0/T8, inferred
    from the APs' base partitions) execute concurrently.
  - exp: split across Scalar (exact ACT Exp, bf16 out) and Vector
    (Schraudolph: et_bits = int16(round(s*23.083 + 16250.5)) viewed as
    bf16 ~= exp(0.125 s); ~2% elementwise, used on 16/64 tiles).
  - AV transposed form: out^T[d, q] = V_j^T @ E^T.  V block [128k, 65]
    (V|1 with exp(mask) folded; ones col -> denominator row 64) is the
    STATIONARY operand (65-col LDWEIGHTS instead of the old 128-col E^T
    loads), E^T slots are the moving operand (N=128/matmul).  Two
    accumulating matmuls per query block into av[65, ...] PSUM.
  - Evacuation: one DVE tensor_scalar copy [65, 1024] per (pair, group)
    PSUM fp32 -> SBUF bf16, DMA'd as [65, 2 heads, 512 cols] into
    o[65, HPC, T].  Final normalize (num/den) + transpose on host.
"""

import numpy as np
import ml_dtypes
from contextlib import ExitStack

import concourse.mybir as mybir
import concourse.tile as tile
from concourse import bacc
from concourse.bass_utils import run_bass_kernel_spmd

B, H, T, D = 4, 16, 4096, 64
W = 128                       # query/key block width
NCORES = 8
HPC = (B * H) // NCORES       # heads per core = 8
NPAIR = HPC // 2              # head pairs per core = 4
GRP = 4                       # query blocks per exp/evac group
NBLK = T // W
BF16 = mybir.dt.bfloat16
FP32 = mybir.dt.float32
I16 = mybir.dt.int16

# Schraudolph bf16-exp: bits = round(x * 0.125 * 128/ln2 + (127*128 - 5.5))
SCH_A = 0.125 * 128.0 / float(np.log(2.0))
SCH_B = 127.0 * 128.0 - 5.5

# Slot permutation inside one group's [128, 8, 128] score tile (key-major):
# SLOT[bi] = (slot of half j=g0+bi-1, slot of half j=g0+bi).
SLOT = [(0, 1), (2, 4), (5, 6), (7, 3)]
# Score matmuls: (key offset dk from g0, first slot, n_query_blocks)
SMM = [(-1, 0, 1), (0, 1, 2), (1, 4, 2), (2, 6, 2), (3, 3, 1)]


def build_nc(t=T, npair=NPAIR, grp=GRP):
    """Build the single-core Bass program (SPMD across 8 cores)."""
    nblk = t // W
    ngrp = nblk // grp
    hpc = npair * 2
    nc = bacc.Bacc("TRN2", debug=False, enable_asserts=False)
    qtd = nc.dram_tensor("qt", [npair * W, t], BF16, kind="ExternalInput").ap()
    ktd = nc.dram_tensor("kt", [npair * W, t], BF16, kind="ExternalInput").ap()
    vod = nc.dram_tensor("vo", [hpc * W, nblk * (D + 1)], BF16,
                         kind="ExternalInput").ap()
    ood = nc.dram_tensor("o", [D + 1, hpc, t], BF16, kind="ExternalOutput").ap()

    Exp = mybir.ActivationFunctionType.Exp
    Mult = mybir.AluOpType.mult
    Add = mybir.AluOpType.add
    with tile.TileContext(nc) as tc, ExitStack() as ctx:
        qk = ctx.enter_context(tc.tile_pool(name="qk", bufs=2))
        vp = ctx.enter_context(tc.tile_pool(name="vp", bufs=2))
        ep = ctx.enter_context(tc.tile_pool(name="ep", bufs=3))
        obp = ctx.enter_context(tc.tile_pool(name="obp", bufs=4))
        stp = ctx.enter_context(tc.tile_pool(name="stp", bufs=1, space="PSUM"))
        avp = ctx.enter_context(tc.tile_pool(name="avp", bufs=2, space="PSUM"))

        for pr in range(npair):
            qt = qk.tile([W, t], BF16, tag="qt")
            kt = qk.tile([W, t], BF16, tag="kt")
            # split big input DMAs so the first matmuls start sooner
            th = t // 2
            nc.sync.dma_start(out=qt[:, 0:th], in_=qtd[pr * W:(pr + 1) * W, 0:th])
            nc.sync.dma_start(out=kt[:, 0:th], in_=ktd[pr * W:(pr + 1) * W, 0:th])
            nc.sync.dma_start(out=qt[:, th:t], in_=qtd[pr * W:(pr + 1) * W, th:t])
            nc.sync.dma_start(out=kt[:, th:t], in_=ktd[pr * W:(pr + 1) * W, th:t])
            vts = []
            for e in range(2):
                h = 2 * pr + e
                vt = vp.tile([W, nblk, D + 1], BF16, tag=f"v{e}", name=f"vt{e}")
                nc.sync.dma_start(
                    out=vt[:],
                    in_=vod[h * W:(h + 1) * W, :].rearrange(
                        "p (n d) -> p n d", d=D + 1),
                )
                vts.append(vt)

            # software pipeline state: (ets, g) of the previous group, and
            # (av, g) pending evacuation.
            prev = None
            pend = None
            for g in range(ngrp + 2):
                if g < ngrp:
                    g0 = g * grp
                    sts, ets_ = [], []
                    for e in range(2):
                        st = stp.tile([W, 2 * grp, W], FP32, tag=f"st{e}",
                                      name=f"st{e}")
                        if g == 0:
                            # slot (block 0, half j=-1) never read; keep finite
                            nc.vector.memset(st[:, 0, :], 0.0)
                        sts.append(st)
                    # interleave the two heads' matmuls: their (64,128) row
                    # tiles at base partitions 0/64 execute concurrently
                    for dk, s0, nq in SMM:
                        j = g0 + dk            # key block
                        if j < 0:
                            continue
                        qb0 = g0 if dk == -1 else j
                        for e in range(2):
                            dsl = slice(D * e, D * (e + 1))
                            nc.tensor.matmul(
                                sts[e][:, s0:s0 + nq, :],
                                kt[dsl, W * j:W * (j + 1)],
                                qt[dsl, W * qb0:W * (qb0 + nq)],
                                start=True, stop=True,
                            )
                    for e in range(2):
                        et = ep.tile([W, 2 * grp, W], BF16, tag=f"et{e}",
                                     name=f"et{e}")
                        if e == 1 and (g % 2 == 1):
                            # Schraudolph exp on DVE (approx, offloads Scalar)
                            nc.vector.tensor_scalar(
                                out=et[:].bitcast(I16), in0=sts[e][:],
                                scalar1=SCH_A, scalar2=SCH_B,
                                op0=Mult, op1=Add,
                            )
                        else:
                            nc.scalar.activation(et[:], sts[e][:], Exp,
                                                 scale=0.125)
                        ets_.append(et)

                # AV for the previous group (gives exp time to finish)
                if prev is not None:
                    pets, pg = prev
                    pg0 = pg * grp
                    av = avp.tile([D + 1, 2, grp * W], FP32, tag="av",
                                  name="av")
                    for e in range(2):
                        for bi in range(grp):
                            i = pg0 + bi
                            mms = [(SLOT[bi][hi], j)
                                   for hi, j in enumerate((i - 1, i)) if j >= 0]
                            for x, (s, j) in enumerate(mms):
                                nc.tensor.matmul(
                                    av[:, e, W * bi:W * (bi + 1)],
                                    vts[e][:, j, :],
                                    pets[e][:, s, :],
                                    start=(x == 0), stop=(x == len(mms) - 1),
                                )
                    pend_new = (av, pg)
                else:
                    pend_new = None

                # evacuate the AV group finished last iteration
                if pend is not None:
                    pav, eg = pend
                    ob = obp.tile([D + 1, 2, grp * W], BF16, tag="ob",
                                  name="ob")
                    nc.vector.tensor_scalar(
                        out=ob[:], in0=pav[:], scalar1=1.0, scalar2=None,
                        op0=Mult,
                    )
                    nc.sync.dma_start(
                        out=ood[:, 2 * pr:2 * pr + 2,
                                eg * grp * W:(eg + 1) * grp * W],
                        in_=ob[:],
                    )

                pend = pend_new
                if g < ngrp:
                    prev = (ets_, g)
                else:
                    prev = None
    nc.compile()
    return nc


_NC = None


def _get_nc():
    global _NC
    if _NC is None:
        _NC = build_nc()
    return _NC


def make_in_maps(query_layer, key_layer, value_layer, attention_mask):
    q = np.asarray(query_layer)
    k = np.asarray(key_layer)
    v = np.asarray(value_layer)
    m = np.asarray(attention_mask, dtype=np.float32)
    bf = ml_dtypes.bfloat16
    qf = q.reshape(B * H, T, D)
    kf = k.reshape(B * H, T, D)
    em = np.exp(m)                                   # [B, T] per-key mask factor
    in_maps = []
    for c in range(NCORES):
        sl = slice(c * HPC, (c + 1) * HPC)
        b = (c * HPC) // H
        qc = (qf[sl].astype(bf).reshape(NPAIR, 2, T, D)
              .transpose(0, 1, 3, 2).reshape(NPAIR * W, T))
        kc = (kf[sl].astype(bf).reshape(NPAIR, 2, T, D)
              .transpose(0, 1, 3, 2).reshape(NPAIR * W, T))
        vc = np.empty((HPC, T, D + 1), np.float32)
        vc[:, :, :D] = v.reshape(B * H, T, D)[sl] * em[b][None, :, None]
        vc[:, :, D] = em[b][None, :]
        voc = (vc.astype(bf).reshape(HPC, NBLK, W, D + 1)
               .transpose(0, 2, 1, 3).reshape(HPC * W, NBLK * (D + 1)))
        in_maps.append({
            "qt": np.ascontiguousarray(qc),
            "kt": np.ascontiguousarray(kc),
            "vo": np.ascontiguousarray(voc),
        })
    return in_maps


def run(inputs, trace=False):
    """Run on the 8 cores; returns (full_output, BassKernelResults)."""
    in_maps = make_in_maps(**inputs)
    nc = _get_nc()
    res = run_bass_kernel_spmd(
        nc, in_maps, core_ids=list(range(NCORES)), trace=trace
    )
    out = np.empty((B * H, T, D), np.float32)
    for c in range(NCORES):
        oc = res.results[c]["o"].astype(np.float32)     # [65, HPC, T]
        num = oc[:D]                                    # [64, HPC, T]
        den = oc[D]                                     # [HPC, T]
        out[c * HPC:(c + 1) * HPC] = (num / den[None]).transpose(1, 2, 0)
    return out.reshape(B, H, T, D), res


def kernel(query_layer, key_layer, value_layer, attention_mask):
    out, _ = run({
        "query_layer": query_layer,
        "key_layer": key_layer,
        "value_layer": value_layer,
        "attention_mask": attention_mask,
    })
    return out


# revision 5
# speedup vs baseline: 1.0361x; 1.0361x over previous
"""Sliding-window local attention (KeOps ranges) on 8 Trainium2 cores.

Problem: B=4 H=16 T=4096 D=64, query block w=128 attends keys
[128(i-1), 128(i+1)) clamped to [0, T).  Softmax over the 256-key window,
out = attn @ V.  Only block 0 has out-of-range keys (its lower half), so
masking reduces to skipping that half-block.

Sharding: batch*head (64 pairs) split across 8 cores, 8 heads per core.

v2 design (per-core, all matmuls bf16, fp32 PSUM):
  - Scores S^T[k, q] = K_blk @ Q_blk^T, key-major, d=64 contraction on
    partitions [0:64] (head A) / [64:128] (head B).  The two heads' matmuls
    are emitted INTERLEAVED so the PE's 64x128 row-tiles (T0/T8, inferred
    from the APs' base partitions) execute concurrently.
  - exp: split across Scalar (exact ACT Exp, bf16 out) and Vector
    (Schraudolph: et_bits = int16(round(s*23.083 + 16250.5)) viewed as
    bf16 ~= exp(0.125 s); ~2% elementwise, used on 16/64 tiles).
  - AV transposed form: out^T[d, q] = V_j^T @ E^T.  V block [128k, 65]
    (V|1 with exp(mask) folded; ones col -> denominator row 64) is the
    STATIONARY operand (65-col LDWEIGHTS instead of the old 128-col E^T
    loads), E^T slots are the moving operand (N=128/matmul).  Two
    accumulating matmuls per query block into av[65, ...] PSUM.
  - Evacuation: one DVE tensor_scalar copy [65, 1024] per (pair, group)
    PSUM fp32 -> SBUF bf16, DMA'd as [65, 2 heads, 512 cols] into
    o[65, HPC, T].  Final normalize (num/den) + transpose on host.
"""

import numpy as np
import ml_dtypes
from contextlib import ExitStack

import concourse.mybir as mybir
import concourse.tile as tile
from concourse import bacc
from concourse.bass_utils import run_bass_kernel_spmd

B, H, T, D = 4, 16, 4096, 64
W = 128                       # query/key block width
NCORES = 8
HPC = (B * H) // NCORES       # heads per core = 8
NPAIR = HPC // 2              # head pairs per core = 4
GRP = 4                       # query blocks per exp/evac group
NBLK = T // W
BF16 = mybir.dt.bfloat16
FP32 = mybir.dt.float32
I16 = mybir.dt.int16

# Schraudolph bf16-exp: bits = round(x * 0.125 * 128/ln2 + (127*128 - 5.5))
SCH_A = 0.125 * 128.0 / float(np.log(2.0))
SCH_B = 127.0 * 128.0 - 5.5

# Slot permutation inside one group's [128, 8, 128] score tile (key-major):
# SLOT[bi] = (slot of half j=g0+bi-1, slot of half j=g0+bi).
SLOT = [(0, 1), (2, 4), (5, 6), (7, 3)]
# Score matmuls: (key offset dk from g0, first slot, n_query_blocks)
SMM = [(-1, 0, 1), (0, 1, 2), (1, 4, 2), (2, 6, 2), (3, 3, 1)]


def build_nc(t=T, npair=NPAIR, grp=GRP):
    """Build the single-core Bass program (SPMD across 8 cores)."""
    nblk = t // W
    ngrp = nblk // grp
    hpc = npair * 2
    nc = bacc.Bacc("TRN2", debug=False, enable_asserts=False)
    qtd = nc.dram_tensor("qt", [npair * W, t], BF16, kind="ExternalInput").ap()
    ktd = nc.dram_tensor("kt", [npair * W, t], BF16, kind="ExternalInput").ap()
    vod = nc.dram_tensor("vo", [hpc * W, nblk * (D + 1)], BF16,
                         kind="ExternalInput").ap()
    ood = nc.dram_tensor("o", [D + 1, hpc, t], BF16, kind="ExternalOutput").ap()

    Exp = mybir.ActivationFunctionType.Exp
    Mult = mybir.AluOpType.mult
    Add = mybir.AluOpType.add
    with tile.TileContext(nc) as tc, ExitStack() as ctx:
        qk = ctx.enter_context(tc.tile_pool(name="qk", bufs=2))
        vp = ctx.enter_context(tc.tile_pool(name="vp", bufs=2))
        ep = ctx.enter_context(tc.tile_pool(name="ep", bufs=3))
        obp = ctx.enter_context(tc.tile_pool(name="obp", bufs=4))
        # PSUM budget (8 banks of 2KB): st0 x2 bufs (4) + st1 (2) + av (2)
        st0p = ctx.enter_context(tc.tile_pool(name="st0p", bufs=2, space="PSUM"))
        st1p = ctx.enter_context(tc.tile_pool(name="st1p", bufs=1, space="PSUM"))
        avp = ctx.enter_context(tc.tile_pool(name="avp", bufs=1, space="PSUM"))

        for pr in range(npair):
            qt = qk.tile([W, t], BF16, tag="qt")
            kt = qk.tile([W, t], BF16, tag="kt")
            # split big input DMAs so the first matmuls start sooner
            th = t // 2
            nc.sync.dma_start(out=qt[:, 0:th], in_=qtd[pr * W:(pr + 1) * W, 0:th])
            nc.sync.dma_start(out=kt[:, 0:th], in_=ktd[pr * W:(pr + 1) * W, 0:th])
            nc.sync.dma_start(out=qt[:, th:t], in_=qtd[pr * W:(pr + 1) * W, th:t])
            nc.sync.dma_start(out=kt[:, th:t], in_=ktd[pr * W:(pr + 1) * W, th:t])
            vts = []
            for e in range(2):
                h = 2 * pr + e
                vt = vp.tile([W, nblk, D + 1], BF16, tag=f"v{e}", name=f"vt{e}")
                nc.sync.dma_start(
                    out=vt[:],
                    in_=vod[h * W:(h + 1) * W, :].rearrange(
                        "p (n d) -> p n d", d=D + 1),
                )
                vts.append(vt)

            # software pipeline state: (ets, g) of the previous group, and
            # (av, g) pending evacuation.
            prev = None
            pend = None
            for g in range(ngrp + 2):
                if g < ngrp:
                    g0 = g * grp
                    sts, ets_ = [], []
                    for e in range(2):
                        pool = st0p if e == 0 else st1p
                        st = pool.tile([W, 2 * grp, W], FP32, tag=f"st{e}",
                                       name=f"st{e}")
                        if g == 0:
                            # slot (block 0, half j=-1) never read; keep finite
                            nc.vector.memset(st[:, 0, :], 0.0)
                        sts.append(st)
                    # interleave the two heads' matmuls: their (64,128) row
                    # tiles at base partitions 0/64 execute concurrently
                    for dk, s0, nq in SMM:
                        j = g0 + dk            # key block
                        if j < 0:
                            continue
                        qb0 = g0 if dk == -1 else j
                        for e in range(2):
                            dsl = slice(D * e, D * (e + 1))
                            nc.tensor.matmul(
                                sts[e][:, s0:s0 + nq, :],
                                kt[dsl, W * j:W * (j + 1)],
                                qt[dsl, W * qb0:W * (qb0 + nq)],
                                start=True, stop=True,
                            )
                    for e in range(2):
                        et = ep.tile([W, 2 * grp, W], BF16, tag=f"et{e}",
                                     name=f"et{e}")
                        if e == 0 and (g % 2 == 1):
                            # Schraudolph exp on DVE (approx, offloads Scalar).
                            # e=1 (single PSUM buf) always takes the Scalar
                            # path so its WAR releases promptly.
                            nc.vector.tensor_scalar(
                                out=et[:].bitcast(I16), in0=sts[e][:],
                                scalar1=SCH_A, scalar2=SCH_B,
                                op0=Mult, op1=Add,
                            )
                        else:
                            nc.scalar.activation(et[:], sts[e][:], Exp,
                                                 scale=0.125)
                        ets_.append(et)

                # AV for the previous group (gives exp time to finish)
                if prev is not None:
                    pets, pg = prev
                    pg0 = pg * grp
                    av = avp.tile([D + 1, 2, grp * W], FP32, tag="av",
                                  name="av")
                    for e in range(2):
                        last_w = None
                        for bi in range(grp):
                            i = pg0 + bi
                            mms = [(SLOT[bi][hi], j)
                                   for hi, j in enumerate((i - 1, i)) if j >= 0]
                            for x, (s, j) in enumerate(mms):
                                # explicit ldweights, skipped when the same V
                                # block is already resident (adjacent matmuls
                                # for key j share it -> no reload, no drain)
                                if j != last_w:
                                    nc.tensor.ldweights(vts[e][:, j, :])
                                    last_w = j
                                nc.tensor.matmul(
                                    av[:, e, W * bi:W * (bi + 1)],
                                    vts[e][:, j, :],
                                    pets[e][:, s, :],
                                    start=(x == 0), stop=(x == len(mms) - 1),
                                )
                    pend_new = (av, pg)
                else:
                    pend_new = None

                # evacuate the AV group finished last iteration
                if pend is not None:
                    pav, eg = pend
                    ob = obp.tile([D + 1, 2, grp * W], BF16, tag="ob",
                                  name="ob")
                    nc.vector.tensor_scalar(
                        out=ob[:], in0=pav[:], scalar1=1.0, scalar2=None,
                        op0=Mult,
                    )
                    nc.sync.dma_start(
                        out=ood[:, 2 * pr:2 * pr + 2,
                                eg * grp * W:(eg + 1) * grp * W],
                        in_=ob[:],
                    )

                pend = pend_new
                if g < ngrp:
                    prev = (ets_, g)
                else:
                    prev = None
    nc.compile()
    return nc


_NC = None


def _get_nc():
    global _NC
    if _NC is None:
        _NC = build_nc()
    return _NC


def make_in_maps(query_layer, key_layer, value_layer, attention_mask):
    q = np.asarray(query_layer)
    k = np.asarray(key_layer)
    v = np.asarray(value_layer)
    m = np.asarray(attention_mask, dtype=np.float32)
    bf = ml_dtypes.bfloat16
    qf = q.reshape(B * H, T, D)
    kf = k.reshape(B * H, T, D)
    em = np.exp(m)                                   # [B, T] per-key mask factor
    in_maps = []
    for c in range(NCORES):
        sl = slice(c * HPC, (c + 1) * HPC)
        b = (c * HPC) // H
        qc = (qf[sl].astype(bf).reshape(NPAIR, 2, T, D)
              .transpose(0, 1, 3, 2).reshape(NPAIR * W, T))
        kc = (kf[sl].astype(bf).reshape(NPAIR, 2, T, D)
              .transpose(0, 1, 3, 2).reshape(NPAIR * W, T))
        vc = np.empty((HPC, T, D + 1), np.float32)
        vc[:, :, :D] = v.reshape(B * H, T, D)[sl] * em[b][None, :, None]
        vc[:, :, D] = em[b][None, :]
        voc = (vc.astype(bf).reshape(HPC, NBLK, W, D + 1)
               .transpose(0, 2, 1, 3).reshape(HPC * W, NBLK * (D + 1)))
        in_maps.append({
            "qt": np.ascontiguousarray(qc),
            "kt": np.ascontiguousarray(kc),
            "vo": np.ascontiguousarray(voc),
        })
    return in_maps


def run(inputs, trace=False):
    """Run on the 8 cores; returns (full_output, BassKernelResults)."""
    in_maps = make_in_maps(**inputs)
    nc = _get_nc()
    res = run_bass_kernel_spmd(
        nc, in_maps, core_ids=list(range(NCORES)), trace=trace
    )
    out = np.empty((B * H, T, D), np.float32)
    for c in range(NCORES):
        oc = res.results[c]["o"].astype(np.float32)     # [65, HPC, T]
        num = oc[:D]                                    # [64, HPC, T]
        den = oc[D]                                     # [HPC, T]
        out[c * HPC:(c + 1) * HPC] = (num / den[None]).transpose(1, 2, 0)
    return out.reshape(B, H, T, D), res


def kernel(query_layer, key_layer, value_layer, attention_mask):
    out, _ = run({
        "query_layer": query_layer,
        "key_layer": key_layer,
        "value_layer": value_layer,
        "attention_mask": attention_mask,
    })
    return out


# revision 9
# speedup vs baseline: 1.4245x; 1.3749x over previous
"""Sliding-window local attention (KeOps ranges) on 8 Trainium2 cores.

Problem: B=4 H=16 T=4096 D=64, query block w=128 attends keys
[128(i-1), 128(i+1)) clamped to [0, T).  Softmax over the 256-key window,
out = attn @ V.  Only block 0 has out-of-range keys (its lower half), so
masking reduces to skipping that half-block.

Sharding: batch*head (64 pairs) split across 8 cores, 8 heads per core.

v2 design (per-core, all matmuls bf16, fp32 PSUM):
  - Scores S^T[k, q] = K_blk @ Q_blk^T, key-major, d=64 contraction on
    partitions [0:64] (head A) / [64:128] (head B).  The two heads' matmuls
    are emitted INTERLEAVED so the PE's 64x128 row-tiles (T0/T8, inferred
    from the APs' base partitions) execute concurrently.
  - exp: split across Scalar (exact ACT Exp, bf16 out) and Vector
    (Schraudolph: et_bits = int16(round(s*23.083 + 16250.5)) viewed as
    bf16 ~= exp(0.125 s); ~2% elementwise, used on 16/64 tiles).
  - AV transposed form: out^T[d, q] = V_j^T @ E^T.  V block [128k, 65]
    (V|1 with exp(mask) folded; ones col -> denominator row 64) is the
    STATIONARY operand (65-col LDWEIGHTS instead of the old 128-col E^T
    loads), E^T slots are the moving operand (N=128/matmul).  Two
    accumulating matmuls per query block into av[65, ...] PSUM.
  - Evacuation: one DVE tensor_scalar copy [65, 1024] per (pair, group)
    PSUM fp32 -> SBUF bf16, DMA'd as [65, 2 heads, 512 cols] into
    o[65, HPC, T].  Final normalize (num/den) + transpose on host.
"""

import numpy as np
import ml_dtypes
from contextlib import ExitStack

import concourse.mybir as mybir
import concourse.tile as tile
from concourse import bacc
from concourse.bass_utils import run_bass_kernel_spmd

B, H, T, D = 4, 16, 4096, 64
W = 128                       # query/key block width
NCORES = 8
HPC = (B * H) // NCORES       # heads per core = 8
NPAIR = HPC // 2              # head pairs per core = 4
GRP = 4                       # query blocks per exp/evac group
NBLK = T // W
BF16 = mybir.dt.bfloat16
FP32 = mybir.dt.float32
I16 = mybir.dt.int16

# Schraudolph bf16-exp: bits = round(x * 0.125 * 128/ln2 + (127*128 - 5.5))
SCH_A = 0.125 * 128.0 / float(np.log(2.0))
SCH_B = 127.0 * 128.0 - 5.5

# Slot permutation inside one group's [128, 8, 128] score tile (key-major):
# SLOT[bi] = (slot of half j=g0+bi-1, slot of half j=g0+bi).
SLOT = [(0, 1), (2, 4), (5, 6), (7, 3)]
# Score matmuls: (key offset dk from g0, first slot, n_query_blocks)
SMM = [(-1, 0, 1), (0, 1, 2), (1, 4, 2), (2, 6, 2), (3, 3, 1)]


def build_nc(t=T, npair=NPAIR, grp=GRP):
    """Build the single-core Bass program (SPMD across 8 cores)."""
    nblk = t // W
    ngrp = nblk // grp
    hpc = npair * 2
    nc = bacc.Bacc("TRN2", debug=False, enable_asserts=False)
    qtd = nc.dram_tensor("qt", [npair * W, t], BF16, kind="ExternalInput").ap()
    ktd = nc.dram_tensor("kt", [npair * W, t], BF16, kind="ExternalInput").ap()
    vod = nc.dram_tensor("vo", [hpc * W, nblk * (D + 1)], BF16,
                         kind="ExternalInput").ap()
    ood = nc.dram_tensor("o", [D + 1, hpc, t], BF16, kind="ExternalOutput").ap()

    Exp = mybir.ActivationFunctionType.Exp
    Mult = mybir.AluOpType.mult
    Add = mybir.AluOpType.add
    with tile.TileContext(nc) as tc, ExitStack() as ctx:
        qk = ctx.enter_context(tc.tile_pool(name="qk", bufs=2))
        vp = ctx.enter_context(tc.tile_pool(name="vp", bufs=2))
        ep = ctx.enter_context(tc.tile_pool(name="ep", bufs=3))
        obp = ctx.enter_context(tc.tile_pool(name="obp", bufs=4))
        # PSUM budget (8 banks of 2KB): st0 x2 bufs (4 banks) + st1 x2 (4).
        # The AV accumulator aliases the group's consumed st0 tile (its exp
        # has read it by then), so no separate av pool is needed.
        st0p = ctx.enter_context(tc.tile_pool(name="st0p", bufs=2, space="PSUM"))
        st1p = ctx.enter_context(tc.tile_pool(name="st1p", bufs=2, space="PSUM"))

        for pr in range(npair):
            qt = qk.tile([W, t], BF16, tag="qt")
            kt = qk.tile([W, t], BF16, tag="kt")
            # split big input DMAs so the first matmuls start sooner
            th = t // 2
            nc.sync.dma_start(out=qt[:, 0:th], in_=qtd[pr * W:(pr + 1) * W, 0:th])
            nc.sync.dma_start(out=kt[:, 0:th], in_=ktd[pr * W:(pr + 1) * W, 0:th])
            nc.sync.dma_start(out=qt[:, th:t], in_=qtd[pr * W:(pr + 1) * W, th:t])
            nc.sync.dma_start(out=kt[:, th:t], in_=ktd[pr * W:(pr + 1) * W, th:t])
            vts = []
            for e in range(2):
                h = 2 * pr + e
                vt = vp.tile([W, nblk, D + 1], BF16, tag=f"v{e}", name=f"vt{e}")
                nc.sync.dma_start(
                    out=vt[:],
                    in_=vod[h * W:(h + 1) * W, :].rearrange(
                        "p (n d) -> p n d", d=D + 1),
                )
                vts.append(vt)

            # software pipeline state: (ets, g) of the previous group, and
            # (av, g) pending evacuation.
            prev = None
            pend = None
            for g in range(ngrp + 2):
                if g < ngrp:
                    g0 = g * grp
                    sts, ets_ = [], []
                    for e in range(2):
                        pool = st0p if e == 0 else st1p
                        st = pool.tile([W, 2 * grp, W], FP32, tag=f"st{e}",
                                       name=f"st{e}")
                        if g == 0:
                            # slot (block 0, half j=-1) never read; keep finite
                            nc.vector.memset(st[:, 0, :], 0.0)
                        sts.append(st)
                    # interleave the two heads' matmuls: their (64,128) row
                    # tiles at base partitions 0/64 execute concurrently
                    for dk, s0, nq in SMM:
                        j = g0 + dk            # key block
                        if j < 0:
                            continue
                        qb0 = g0 if dk == -1 else j
                        for e in range(2):
                            dsl = slice(D * e, D * (e + 1))
                            nc.tensor.matmul(
                                sts[e][:, s0:s0 + nq, :],
                                kt[dsl, W * j:W * (j + 1)],
                                qt[dsl, W * qb0:W * (qb0 + nq)],
                                start=True, stop=True,
                            )
                    for e in range(2):
                        et = ep.tile([W, 2 * grp, W], BF16, tag=f"et{e}",
                                     name=f"et{e}")
                        if e == 0 and (g % 2 == 1):
                            # Schraudolph exp on DVE (approx, offloads Scalar)
                            nc.vector.tensor_scalar(
                                out=et[:].bitcast(I16), in0=sts[e][:],
                                scalar1=SCH_A, scalar2=SCH_B,
                                op0=Mult, op1=Add,
                            )
                        else:
                            nc.scalar.activation(et[:], sts[e][:], Exp,
                                                 scale=0.125)
                        ets_.append(et)

                # AV for the previous group (gives exp time to finish)
                if prev is not None:
                    pets, pst0, pg = prev
                    pg0 = pg * grp
                    # AV accumulator aliases the consumed e0 score tile:
                    # [65, 2 heads, 512] over its [128, 1024] fp32 region
                    av = (pst0[0:D + 1, :, :]
                          .rearrange("p a b -> p (a b)")
                          .rearrange("p (e q) -> p e q", e=2))
                    for e in range(2):
                        for bi in range(grp):
                            i = pg0 + bi
                            mms = [(SLOT[bi][hi], j)
                                   for hi, j in enumerate((i - 1, i)) if j >= 0]
                            for x, (s, j) in enumerate(mms):
                                nc.tensor.matmul(
                                    av[:, e, W * bi:W * (bi + 1)],
                                    vts[e][:, j, :],
                                    pets[e][:, s, :],
                                    start=(x == 0), stop=(x == len(mms) - 1),
                                )
                    pend_new = (av, pg)
                else:
                    pend_new = None

                # evacuate the AV group finished last iteration
                if pend is not None:
                    pav, eg = pend
                    ob = obp.tile([D + 1, 2, grp * W], BF16, tag="ob",
                                  name="ob")
                    nc.vector.tensor_scalar(
                        out=ob[:], in0=pav[:], scalar1=1.0, scalar2=None,
                        op0=Mult,
                    )
                    nc.sync.dma_start(
                        out=ood[:, 2 * pr:2 * pr + 2,
                                eg * grp * W:(eg + 1) * grp * W],
                        in_=ob[:],
                    )

                pend = pend_new
                if g < ngrp:
                    prev = (ets_, sts[0], g)
                else:
                    prev = None
    nc.compile()
    return nc


_NC = None


def _get_nc():
    global _NC
    if _NC is None:
        _NC = build_nc()
    return _NC


def make_in_maps(query_layer, key_layer, value_layer, attention_mask):
    q = np.asarray(query_layer)
    k = np.asarray(key_layer)
    v = np.asarray(value_layer)
    m = np.asarray(attention_mask, dtype=np.float32)
    bf = ml_dtypes.bfloat16
    qf = q.reshape(B * H, T, D)
    kf = k.reshape(B * H, T, D)
    em = np.exp(m)                                   # [B, T] per-key mask factor
    in_maps = []
    for c in range(NCORES):
        sl = slice(c * HPC, (c + 1) * HPC)
        b = (c * HPC) // H
        qc = (qf[sl].astype(bf).reshape(NPAIR, 2, T, D)
              .transpose(0, 1, 3, 2).reshape(NPAIR * W, T))
        kc = (kf[sl].astype(bf).reshape(NPAIR, 2, T, D)
              .transpose(0, 1, 3, 2).reshape(NPAIR * W, T))
        vc = np.empty((HPC, T, D + 1), np.float32)
        vc[:, :, :D] = v.reshape(B * H, T, D)[sl] * em[b][None, :, None]
        vc[:, :, D] = em[b][None, :]
        voc = (vc.astype(bf).reshape(HPC, NBLK, W, D + 1)
               .transpose(0, 2, 1, 3).reshape(HPC * W, NBLK * (D + 1)))
        in_maps.append({
            "qt": np.ascontiguousarray(qc),
            "kt": np.ascontiguousarray(kc),
            "vo": np.ascontiguousarray(voc),
        })
    return in_maps


def run(inputs, trace=False):
    """Run on the 8 cores; returns (full_output, BassKernelResults)."""
    in_maps = make_in_maps(**inputs)
    nc = _get_nc()
    res = run_bass_kernel_spmd(
        nc, in_maps, core_ids=list(range(NCORES)), trace=trace
    )
    out = np.empty((B * H, T, D), np.float32)
    for c in range(NCORES):
        oc = res.results[c]["o"].astype(np.float32)     # [65, HPC, T]
        num = oc[:D]                                    # [64, HPC, T]
        den = oc[D]                                     # [HPC, T]
        out[c * HPC:(c + 1) * HPC] = (num / den[None]).transpose(1, 2, 0)
    return out.reshape(B, H, T, D), res


def kernel(query_layer, key_layer, value_layer, attention_mask):
    out, _ = run({
        "query_layer": query_layer,
        "key_layer": key_layer,
        "value_layer": value_layer,
        "attention_mask": attention_mask,
    })
    return out


# revision 11
# speedup vs baseline: 1.4280x; 1.0025x over previous
"""Sliding-window local attention (KeOps ranges) on 8 Trainium2 cores.

Problem: B=4 H=16 T=4096 D=64, query block w=128 attends keys
[128(i-1), 128(i+1)) clamped to [0, T).  Softmax over the 256-key window,
out = attn @ V.  Only block 0 has out-of-range keys (its lower half), so
masking reduces to skipping that half-block.

Sharding: batch*head (64 pairs) split across 8 cores, 8 heads per core.

v2 design (per-core, all matmuls bf16, fp32 PSUM):
  - Scores S^T[k, q] = K_blk @ Q_blk^T, key-major, d=64 contraction on
    partitions [0:64] (head A) / [64:128] (head B).  The two heads' matmuls
    are emitted INTERLEAVED so the PE's 64x128 row-tiles (T0/T8, inferred
    from the APs' base partitions) execute concurrently.
  - exp: split across Scalar (exact ACT Exp, bf16 out) and Vector
    (Schraudolph: et_bits = int16(round(s*23.083 + 16250.5)) viewed as
    bf16 ~= exp(0.125 s); ~2% elementwise, used on 16/64 tiles).
  - AV transposed form: out^T[d, q] = V_j^T @ E^T.  V block [128k, 65]
    (V|1 with exp(mask) folded; ones col -> denominator row 64) is the
    STATIONARY operand (65-col LDWEIGHTS instead of the old 128-col E^T
    loads), E^T slots are the moving operand (N=128/matmul).  Two
    accumulating matmuls per query block into av[65, ...] PSUM.
  - Evacuation: one DVE tensor_scalar copy [65, 1024] per (pair, group)
    PSUM fp32 -> SBUF bf16, DMA'd as [65, 2 heads, 512 cols] into
    o[65, HPC, T].  Final normalize (num/den) + transpose on host.
"""

import numpy as np
import ml_dtypes
from contextlib import ExitStack

import concourse.mybir as mybir
import concourse.tile as tile
from concourse import bacc
from concourse.bass_utils import run_bass_kernel_spmd

B, H, T, D = 4, 16, 4096, 64
W = 128                       # query/key block width
NCORES = 8
HPC = (B * H) // NCORES       # heads per core = 8
NPAIR = HPC // 2              # head pairs per core = 4
GRP = 4                       # query blocks per exp/evac group
NBLK = T // W
BF16 = mybir.dt.bfloat16
FP32 = mybir.dt.float32
I16 = mybir.dt.int16

# Schraudolph bf16-exp: bits = round(x * 0.125 * 128/ln2 + (127*128 - 5.5))
SCH_A = 0.125 * 128.0 / float(np.log(2.0))
SCH_B = 127.0 * 128.0 - 5.5

# Slot permutation inside one group's [128, 8, 128] score tile (key-major):
# SLOT[bi] = (slot of half j=g0+bi-1, slot of half j=g0+bi).
SLOT = [(0, 1), (2, 4), (5, 6), (7, 3)]
# Score matmuls: (key offset dk from g0, first slot, n_query_blocks)
SMM = [(-1, 0, 1), (0, 1, 2), (1, 4, 2), (2, 6, 2), (3, 3, 1)]


def build_nc(t=T, npair=NPAIR, grp=GRP):
    """Build the single-core Bass program (SPMD across 8 cores)."""
    nblk = t // W
    ngrp = nblk // grp
    hpc = npair * 2
    nc = bacc.Bacc("TRN2", debug=False, enable_asserts=False)
    qtd = nc.dram_tensor("qt", [npair * W, t], BF16, kind="ExternalInput").ap()
    ktd = nc.dram_tensor("kt", [npair * W, t], BF16, kind="ExternalInput").ap()
    vod = nc.dram_tensor("vo", [hpc * W, nblk * (D + 1)], BF16,
                         kind="ExternalInput").ap()
    ood = nc.dram_tensor("o", [D + 1, hpc, t], BF16, kind="ExternalOutput").ap()

    Exp = mybir.ActivationFunctionType.Exp
    Mult = mybir.AluOpType.mult
    Add = mybir.AluOpType.add
    with tile.TileContext(nc) as tc, ExitStack() as ctx:
        qk = ctx.enter_context(tc.tile_pool(name="qk", bufs=2))
        vp = ctx.enter_context(tc.tile_pool(name="vp", bufs=2))
        ep = ctx.enter_context(tc.tile_pool(name="ep", bufs=3))
        obp = ctx.enter_context(tc.tile_pool(name="obp", bufs=4))
        # PSUM budget (8 banks of 2KB): st0 x2 bufs (4 banks) + st1 x2 (4).
        # The AV accumulator aliases the group's consumed st0 tile (its exp
        # has read it by then), so no separate av pool is needed.
        st0p = ctx.enter_context(tc.tile_pool(name="st0p", bufs=2, space="PSUM"))
        st1p = ctx.enter_context(tc.tile_pool(name="st1p", bufs=2, space="PSUM"))

        for pr in range(npair):
            qt = qk.tile([W, t], BF16, tag="qt")
            kt = qk.tile([W, t], BF16, tag="kt")
            # split big input DMAs so the first matmuls start sooner
            th = t // 2
            nc.sync.dma_start(out=qt[:, 0:th], in_=qtd[pr * W:(pr + 1) * W, 0:th])
            nc.sync.dma_start(out=kt[:, 0:th], in_=ktd[pr * W:(pr + 1) * W, 0:th])
            nc.sync.dma_start(out=qt[:, th:t], in_=qtd[pr * W:(pr + 1) * W, th:t])
            nc.sync.dma_start(out=kt[:, th:t], in_=ktd[pr * W:(pr + 1) * W, th:t])
            vts = []
            for e in range(2):
                h = 2 * pr + e
                vt = vp.tile([W, nblk, D + 1], BF16, tag=f"v{e}", name=f"vt{e}")
                nc.sync.dma_start(
                    out=vt[:],
                    in_=vod[h * W:(h + 1) * W, :].rearrange(
                        "p (n d) -> p n d", d=D + 1),
                )
                vts.append(vt)

            # software pipeline state: (ets, g) of the previous group, and
            # (av, g) pending evacuation.
            prev = None
            pend = None
            for g in range(ngrp + 2):
                # evacuate the AV group finished last iteration FIRST: it is
                # already ready, and emitting it ahead of exp in the DVE
                # stream releases the aliased st0 buffer sooner (scores g
                # reuse it)
                if pend is not None:
                    pav, eg = pend
                    ob = obp.tile([D + 1, 2, grp * W], BF16, tag="ob",
                                  name="ob")
                    nc.vector.tensor_scalar(
                        out=ob[:], in0=pav[:], scalar1=1.0, scalar2=None,
                        op0=Mult,
                    )
                    nc.sync.dma_start(
                        out=ood[:, 2 * pr:2 * pr + 2,
                                eg * grp * W:(eg + 1) * grp * W],
                        in_=ob[:],
                    )
                    pend = None

                if g < ngrp:
                    g0 = g * grp
                    sts, ets_ = [], []
                    for e in range(2):
                        pool = st0p if e == 0 else st1p
                        st = pool.tile([W, 2 * grp, W], FP32, tag=f"st{e}",
                                       name=f"st{e}")
                        if g == 0:
                            # slot (block 0, half j=-1) never read; keep finite
                            nc.vector.memset(st[:, 0, :], 0.0)
                        sts.append(st)
                    # interleave the two heads' matmuls: their (64,128) row
                    # tiles at base partitions 0/64 execute concurrently
                    for dk, s0, nq in SMM:
                        j = g0 + dk            # key block
                        if j < 0:
                            continue
                        qb0 = g0 if dk == -1 else j
                        for e in range(2):
                            dsl = slice(D * e, D * (e + 1))
                            nc.tensor.matmul(
                                sts[e][:, s0:s0 + nq, :],
                                kt[dsl, W * j:W * (j + 1)],
                                qt[dsl, W * qb0:W * (qb0 + nq)],
                                start=True, stop=True,
                            )
                    for e in range(2):
                        et = ep.tile([W, 2 * grp, W], BF16, tag=f"et{e}",
                                     name=f"et{e}")
                        if e == 0 and (g % 2 == 1):
                            # Schraudolph exp on DVE (approx, offloads Scalar)
                            nc.vector.tensor_scalar(
                                out=et[:].bitcast(I16), in0=sts[e][:],
                                scalar1=SCH_A, scalar2=SCH_B,
                                op0=Mult, op1=Add,
                            )
                        else:
                            nc.scalar.activation(et[:], sts[e][:], Exp,
                                                 scale=0.125)
                        ets_.append(et)

                # AV for the previous group (gives exp time to finish)
                if prev is not None:
                    pets, pst0, pg = prev
                    pg0 = pg * grp
                    # AV accumulator aliases the consumed e0 score tile:
                    # [65, 2 heads, 512] over its [128, 1024] fp32 region
                    av = (pst0[0:D + 1, :, :]
                          .rearrange("p a b -> p (a b)")
                          .rearrange("p (e q) -> p e q", e=2))
                    for e in range(2):
                        for bi in range(grp):
                            i = pg0 + bi
                            mms = [(SLOT[bi][hi], j)
                                   for hi, j in enumerate((i - 1, i)) if j >= 0]
                            for x, (s, j) in enumerate(mms):
                                nc.tensor.matmul(
                                    av[:, e, W * bi:W * (bi + 1)],
                                    vts[e][:, j, :],
                                    pets[e][:, s, :],
                                    start=(x == 0), stop=(x == len(mms) - 1),
                                )
                    pend_new = (av, pg)
                else:
                    pend_new = None

                pend = pend_new
                if g < ngrp:
                    prev = (ets_, sts[0], g)
                else:
                    prev = None
    nc.compile()
    return nc


_NC = None


def _get_nc():
    global _NC
    if _NC is None:
        _NC = build_nc()
    return _NC


def make_in_maps(query_layer, key_layer, value_layer, attention_mask):
    q = np.asarray(query_layer)
    k = np.asarray(key_layer)
    v = np.asarray(value_layer)
    m = np.asarray(attention_mask, dtype=np.float32)
    bf = ml_dtypes.bfloat16
    qf = q.reshape(B * H, T, D)
    kf = k.reshape(B * H, T, D)
    em = np.exp(m)                                   # [B, T] per-key mask factor
    in_maps = []
    for c in range(NCORES):
        sl = slice(c * HPC, (c + 1) * HPC)
        b = (c * HPC) // H
        qc = (qf[sl].astype(bf).reshape(NPAIR, 2, T, D)
              .transpose(0, 1, 3, 2).reshape(NPAIR * W, T))
        kc = (kf[sl].astype(bf).reshape(NPAIR, 2, T, D)
              .transpose(0, 1, 3, 2).reshape(NPAIR * W, T))
        vc = np.empty((HPC, T, D + 1), np.float32)
        vc[:, :, :D] = v.reshape(B * H, T, D)[sl] * em[b][None, :, None]
        vc[:, :, D] = em[b][None, :]
        voc = (vc.astype(bf).reshape(HPC, NBLK, W, D + 1)
               .transpose(0, 2, 1, 3).reshape(HPC * W, NBLK * (D + 1)))
        in_maps.append({
            "qt": np.ascontiguousarray(qc),
            "kt": np.ascontiguousarray(kc),
            "vo": np.ascontiguousarray(voc),
        })
    return in_maps


def run(inputs, trace=False):
    """Run on the 8 cores; returns (full_output, BassKernelResults)."""
    in_maps = make_in_maps(**inputs)
    nc = _get_nc()
    res = run_bass_kernel_spmd(
        nc, in_maps, core_ids=list(range(NCORES)), trace=trace
    )
    out = np.empty((B * H, T, D), np.float32)
    for c in range(NCORES):
        oc = res.results[c]["o"].astype(np.float32)     # [65, HPC, T]
        num = oc[:D]                                    # [64, HPC, T]
        den = oc[D]                                     # [HPC, T]
        out[c * HPC:(c + 1) * HPC] = (num / den[None]).transpose(1, 2, 0)
    return out.reshape(B, H, T, D), res


def kernel(query_layer, key_layer, value_layer, attention_mask):
    out, _ = run({
        "query_layer": query_layer,
        "key_layer": key_layer,
        "value_layer": value_layer,
        "attention_mask": attention_mask,
    })
    return out


# revision 15
# speedup vs baseline: 1.4475x; 1.0137x over previous
"""Sliding-window local attention (KeOps ranges) on 8 Trainium2 cores.

Problem: B=4 H=16 T=4096 D=64, query block w=128 attends keys
[128(i-1), 128(i+1)) clamped to [0, T).  Softmax over the 256-key window,
out = attn @ V.  Only block 0 has out-of-range keys (its lower half), so
masking reduces to skipping that half-block.

Sharding: batch*head (64 pairs) split across 8 cores, 8 heads per core.

v2 design (per-core, all matmuls bf16, fp32 PSUM):
  - Scores S^T[k, q] = K_blk @ Q_blk^T, key-major, d=64 contraction on
    partitions [0:64] (head A) / [64:128] (head B).  The two heads' matmuls
    are emitted INTERLEAVED so the PE's 64x128 row-tiles (T0/T8, inferred
    from the APs' base partitions) execute concurrently.
  - exp: split across Scalar (exact ACT Exp, bf16 out) and Vector
    (Schraudolph: et_bits = int16(round(s*23.083 + 16250.5)) viewed as
    bf16 ~= exp(0.125 s); ~2% elementwise, used on 16/64 tiles).
  - AV transposed form: out^T[d, q] = V_j^T @ E^T.  V block [128k, 65]
    (V|1 with exp(mask) folded; ones col -> denominator row 64) is the
    STATIONARY operand (65-col LDWEIGHTS instead of the old 128-col E^T
    loads), E^T slots are the moving operand (N=128/matmul).  Two
    accumulating matmuls per query block into av[65, ...] PSUM.
  - Evacuation: one DVE tensor_scalar copy [65, 1024] per (pair, group)
    PSUM fp32 -> SBUF bf16, DMA'd as [65, 2 heads, 512 cols] into
    o[65, HPC, T].  Final normalize (num/den) + transpose on host.
"""

import numpy as np
import ml_dtypes
from contextlib import ExitStack

import concourse.mybir as mybir
import concourse.tile as tile
from concourse import bacc
from concourse.bass_utils import run_bass_kernel_spmd

B, H, T, D = 4, 16, 4096, 64
W = 128                       # query/key block width
NCORES = 8
HPC = (B * H) // NCORES       # heads per core = 8
NPAIR = HPC // 2              # head pairs per core = 4
GRP = 4                       # query blocks per exp/evac group
NBLK = T // W
BF16 = mybir.dt.bfloat16
FP32 = mybir.dt.float32
I16 = mybir.dt.int16

# Schraudolph bf16-exp: bits = round(x * 0.125 * 128/ln2 + (127*128 - 5.5))
SCH_A = 0.125 * 128.0 / float(np.log(2.0))
SCH_B = 127.0 * 128.0 - 5.5

# Slot permutation inside one group's [128, 8, 128] score tile (key-major):
# SLOT[bi] = (slot of half j=g0+bi-1, slot of half j=g0+bi).
SLOT = [(0, 1), (2, 4), (5, 6), (7, 3)]
# Score matmuls: (key offset dk from g0, first slot, n_query_blocks)
SMM = [(-1, 0, 1), (0, 1, 2), (1, 4, 2), (2, 6, 2), (3, 3, 1)]


def build_nc(t=T, npair=NPAIR, grp=GRP):
    """Build the single-core Bass program (SPMD across 8 cores)."""
    nblk = t // W
    ngrp = nblk // grp
    hpc = npair * 2
    nc = bacc.Bacc("TRN2", debug=False, enable_asserts=False)
    qtd = nc.dram_tensor("qt", [npair * W, t], BF16, kind="ExternalInput").ap()
    ktd = nc.dram_tensor("kt", [npair * W, t], BF16, kind="ExternalInput").ap()
    vod = nc.dram_tensor("vo", [hpc * W, nblk * (D + 1)], BF16,
                         kind="ExternalInput").ap()
    ood = nc.dram_tensor("o", [D + 1, hpc, t], BF16, kind="ExternalOutput").ap()

    Exp = mybir.ActivationFunctionType.Exp
    Mult = mybir.AluOpType.mult
    Add = mybir.AluOpType.add
    with tile.TileContext(nc) as tc, ExitStack() as ctx:
        qk = ctx.enter_context(tc.tile_pool(name="qk", bufs=2))
        vp = ctx.enter_context(tc.tile_pool(name="vp", bufs=2))
        ep = ctx.enter_context(tc.tile_pool(name="ep", bufs=3))
        obp = ctx.enter_context(tc.tile_pool(name="obp", bufs=4))
        # PSUM budget (8 banks of 2KB): st0 x2 bufs (4 banks) + st1 x2 (4).
        # The AV accumulator aliases the group's consumed st0 tile (its exp
        # has read it by then), so no separate av pool is needed.
        st0p = ctx.enter_context(tc.tile_pool(name="st0p", bufs=2, space="PSUM"))
        st1p = ctx.enter_context(tc.tile_pool(name="st1p", bufs=2, space="PSUM"))

        for pr in range(npair):
            # per-half input tiles: each half is one DMA, so the first
            # groups' matmuls only wait on the first 2MB, not the full load.
            # kt/vo halves overlap by one block (group 16 reads key block 63).
            th = t // 2
            nh = nblk // 2
            rs = slice(pr * W, (pr + 1) * W)
            qth = [qk.tile([W, th], BF16, tag=f"qt{i}", name=f"qt{i}")
                   for i in range(2)]
            kth = [qk.tile([W, th + W], BF16, tag=f"kt{i}", name=f"kt{i}")
                   for i in range(2)]
            nc.sync.dma_start(out=qth[0][:], in_=qtd[rs, 0:th])
            nc.sync.dma_start(out=kth[0][:, 0:th], in_=ktd[rs, 0:th])
            nc.sync.dma_start(out=qth[1][:], in_=qtd[rs, th:t])
            nc.sync.dma_start(out=kth[1][:], in_=ktd[rs, th - W:t])

            def qt_ap(e, b0, nq):
                """Moving slice covering query blocks [b0, b0+nq)."""
                hi = b0 // nh
                off = b0 - hi * nh
                return qth[hi][64 * e:64 * (e + 1), W * off:W * (off + nq)]

            def kt_ap(e, j):
                """Stationary slice for key block j (halves overlap at 63)."""
                hi = 0 if j < nh else 1
                off = (j - nh + 1) if hi else j
                return kth[hi][64 * e:64 * (e + 1), W * off:W * (off + 1)]

            vts = []
            for e in range(2):
                h = 2 * pr + e
                src = vod[h * W:(h + 1) * W, :].rearrange(
                    "p (n d) -> p n d", d=D + 1)
                vt = [vp.tile([W, nh + (i and 1), D + 1], BF16,
                              tag=f"v{e}{i}", name=f"vt{e}{i}")
                      for i in range(2)]
                nc.sync.dma_start(out=vt[0][:], in_=src[:, 0:nh])
                nc.sync.dma_start(out=vt[1][:], in_=src[:, nh - 1:nblk])
                vts.append(vt)

            def vo_ap(e, j):
                hi = 0 if j < nh else 1
                off = (j - nh + 1) if hi else j
                return vts[e][hi][:, off, :]

            # software pipeline state: (ets, g) of the previous group, and
            # (av, g) pending evacuation.
            prev = None
            pend = None
            for g in range(ngrp + 2):
                # evacuate the AV group finished last iteration FIRST: it is
                # already ready, and emitting it ahead of exp in the DVE
                # stream releases the aliased st0 buffer sooner (scores g
                # reuse it)
                if pend is not None:
                    pav, eg = pend
                    ob = obp.tile([D + 1, 2, grp * W], BF16, tag="ob",
                                  name="ob")
                    if eg % 2 == 0:
                        nc.scalar.activation(
                            ob[:], pav[:], mybir.ActivationFunctionType.Copy)
                    else:
                        nc.vector.tensor_scalar(
                            out=ob[:], in0=pav[:], scalar1=1.0, scalar2=None,
                            op0=Mult,
                        )
                    nc.sync.dma_start(
                        out=ood[:, 2 * pr:2 * pr + 2,
                                eg * grp * W:(eg + 1) * grp * W],
                        in_=ob[:],
                    )
                    pend = None

                if g < ngrp:
                    g0 = g * grp
                    sts, ets_ = [], []
                    for e in range(2):
                        pool = st0p if e == 0 else st1p
                        st = pool.tile([W, 2 * grp, W], FP32, tag=f"st{e}",
                                       name=f"st{e}")
                        if g == 0:
                            # slot (block 0, half j=-1) never read; keep finite
                            nc.vector.memset(st[:, 0, :], 0.0)
                        sts.append(st)
                    # interleave the two heads' matmuls: their (64,128) row
                    # tiles at base partitions 0/64 execute concurrently
                    for dk, s0, nq in SMM:
                        j = g0 + dk            # key block
                        if j < 0:
                            continue
                        qb0 = g0 if dk == -1 else j
                        for e in range(2):
                            nc.tensor.matmul(
                                sts[e][:, s0:s0 + nq, :],
                                kt_ap(e, j),
                                qt_ap(e, qb0, nq),
                                start=True, stop=True,
                            )
                    for e in range(2):
                        et = ep.tile([W, 2 * grp, W], BF16, tag=f"et{e}",
                                     name=f"et{e}")
                        if e == 0:
                            # Schraudolph exp on DVE: e0 always, so the two
                            # heads' exps run on different engines in parallel
                            nc.vector.tensor_scalar(
                                out=et[:].bitcast(I16), in0=sts[e][:],
                                scalar1=SCH_A, scalar2=SCH_B,
                                op0=Mult, op1=Add,
                            )
                        else:
                            nc.scalar.activation(et[:], sts[e][:], Exp,
                                                 scale=0.125)
                        ets_.append(et)

                # AV for the previous group (gives exp time to finish)
                if prev is not None:
                    pets, pst0, pg = prev
                    pg0 = pg * grp
                    # AV accumulator aliases the consumed e0 score tile:
                    # [65, 2 heads, 512] over its [128, 1024] fp32 region
                    av = (pst0[0:D + 1, :, :]
                          .rearrange("p a b -> p (a b)")
                          .rearrange("p (e q) -> p e q", e=2))
                    for e in range(2):
                        for bi in range(grp):
                            i = pg0 + bi
                            mms = [(SLOT[bi][hi], j)
                                   for hi, j in enumerate((i - 1, i)) if j >= 0]
                            for x, (s, j) in enumerate(mms):
                                nc.tensor.matmul(
                                    av[:, e, W * bi:W * (bi + 1)],
                                    vo_ap(e, j),
                                    pets[e][:, s, :],
                                    start=(x == 0), stop=(x == len(mms) - 1),
                                )
                    pend_new = (av, pg)
                else:
                    pend_new = None

                pend = pend_new
                if g < ngrp:
                    prev = (ets_, sts[0], g)
                else:
                    prev = None
    nc.compile()
    return nc


_NC = None


def _get_nc():
    global _NC
    if _NC is None:
        _NC = build_nc()
    return _NC


def make_in_maps(query_layer, key_layer, value_layer, attention_mask):
    q = np.asarray(query_layer)
    k = np.asarray(key_layer)
    v = np.asarray(value_layer)
    m = np.asarray(attention_mask, dtype=np.float32)
    bf = ml_dtypes.bfloat16
    qf = q.reshape(B * H, T, D)
    kf = k.reshape(B * H, T, D)
    em = np.exp(m)                                   # [B, T] per-key mask factor
    in_maps = []
    for c in range(NCORES):
        sl = slice(c * HPC, (c + 1) * HPC)
        b = (c * HPC) // H
        qc = (qf[sl].astype(bf).reshape(NPAIR, 2, T, D)
              .transpose(0, 1, 3, 2).reshape(NPAIR * W, T))
        kc = (kf[sl].astype(bf).reshape(NPAIR, 2, T, D)
              .transpose(0, 1, 3, 2).reshape(NPAIR * W, T))
        vc = np.empty((HPC, T, D + 1), np.float32)
        vc[:, :, :D] = v.reshape(B * H, T, D)[sl] * em[b][None, :, None]
        vc[:, :, D] = em[b][None, :]
        voc = (vc.astype(bf).reshape(HPC, NBLK, W, D + 1)
               .transpose(0, 2, 1, 3).reshape(HPC * W, NBLK * (D + 1)))
        in_maps.append({
            "qt": np.ascontiguousarray(qc),
            "kt": np.ascontiguousarray(kc),
            "vo": np.ascontiguousarray(voc),
        })
    return in_maps


def run(inputs, trace=False):
    """Run on the 8 cores; returns (full_output, BassKernelResults)."""
    in_maps = make_in_maps(**inputs)
    nc = _get_nc()
    res = run_bass_kernel_spmd(
        nc, in_maps, core_ids=list(range(NCORES)), trace=trace
    )
    out = np.empty((B * H, T, D), np.float32)
    for c in range(NCORES):
        oc = res.results[c]["o"].astype(np.float32)     # [65, HPC, T]
        num = oc[:D]                                    # [64, HPC, T]
        den = oc[D]                                     # [HPC, T]
        out[c * HPC:(c + 1) * HPC] = (num / den[None]).transpose(1, 2, 0)
    return out.reshape(B, H, T, D), res


def kernel(query_layer, key_layer, value_layer, attention_mask):
    out, _ = run({
        "query_layer": query_layer,
        "key_layer": key_layer,
        "value_layer": value_layer,
        "attention_mask": attention_mask,
    })
    return out
